# revision 4
# baseline (speedup 1.0000x reference)
"""Multi-head attention (B=2, L=2048, d_model=1024, 16 heads) on 8 TRN2 NeuronCores.

Sharding: data-parallel on batch (2) x tensor-parallel on heads (4 groups of 4
heads). Core c handles batch c//4, head group c%4 (Wq/Wk/Wv column-parallel,
Wo row-parallel). Each core emits a partial (2048, 1024) output projection;
the host sums the 4 partials per batch and adds the bias.

Masked keys contribute exactly zero to the reference output (softmax over
-inf), so each data shard compacts K/V to the kept keys (host-side gather,
padded to a multiple of 512; pad slots get zero V rows and ~0 softmax weight).

Device-side pipeline per core (single pass, engines balanced):
  K-proj/Q-proj weight-stationary (few LDWEIGHTS), V-proj natural layout.
  Per 512-query window:
    S^T = K_h Q_h^T per lk-pair into a 4-bank PSUM group (2 heads run on
    disjoint PE row groups, concurrently) -> one exp (ScalarE, N=2048)
    U^T accumulation (lhsT = [V_h | keep], M=65; row 64 = softmax denom)
    denominator rows DMA'd from PSUM to a collect row; reciprocal_approx_fast
    on [1, 2048]; gpsimd partition_broadcast; DVE multiplies -> ctx^T
    out-proj folded per window -> DMA out
"""

import os
import sys

import numpy as np

for _p in ("/opt/trn_rl_repo", "/root/.axon_site/_ro/trn_rl_repo"):
    if os.path.isdir(_p) and _p not in sys.path:
        sys.path.insert(0, _p)

import concourse.bass as bass  # noqa: E402
import concourse.mybir as mybir  # noqa: E402
import concourse.tile as tile  # noqa: E402
from concourse import bacc  # noqa: E402
from concourse import bass_utils  # noqa: E402
from concourse import library_config  # noqa: E402
from concourse.bass_interp import get_hw_module  # noqa: E402

P = 128
D = 1024          # d_model
LQ = 2048         # query length
DH = 256          # per-core head-group width (4 heads x 64)
HD = 64           # head dim
KC = D // P       # contraction chunks for the projections
MC = DH // P      # 2 partition chunks of the head-group dim
NQW = LQ // 512   # query windows
SCALE = 0.125     # 1/sqrt(HD)
F32 = mybir.dt.float32
F16 = mybir.dt.float16
EXP = mybir.ActivationFunctionType.Exp
NCORES = 8
PAD_KEEP = 0.0  # pad keys: V rows zero, keep 0 -> no denominator contribution

_NC_CACHE: dict[int, object] = {}
LAST_RESULTS = None  # test harness reads exec_time_ns off this
TRACE = bool(int(os.environ.get("KERNEL_TRACE", "0")))


def _ensure_ntff_hook():
    """Provide antenv.axon_hooks when the image lacks it (trace-only path)."""
    import importlib
    import types

    try:
        importlib.import_module("antenv.axon_hooks")
        return
    except ImportError:
        pass
    try:
        import antenv
        from trn_agent_boot.trn_boot import _ntff_profile_via_ctypes
    except ImportError:
        return
    mod = types.ModuleType("antenv.axon_hooks")
    state = {"h": None}
    mod.set_axon_ntff_profile_hook = lambda h: state.__setitem__("h", h)
    mod.get_axon_ntff_profile_hook = lambda: state["h"]
    sys.modules["antenv.axon_hooks"] = mod
    antenv.axon_hooks = mod
    so = "/opt/axon/libaxon_pjrt.so"
    if os.path.exists(so):
        mod.set_axon_ntff_profile_hook(_ntff_profile_via_ctypes(so))


def _build(Lkp: int):
    assert Lkp % 512 == 0
    LKC = Lkp // P          # 128-key blocks
    NKW = Lkp // 512        # 512-key windows
    NPAIR = LKC // 2        # lk-pairs per (win, hp)
    nc = bacc.Bacc(
        "TRN2",
        target_bir_lowering=False,
        debug=False,
        enable_asserts=False,
        num_devices=NCORES,
    )

    xq_d = nc.dram_tensor("xq_t", [NQW, P, KC, 512], F16, kind="ExternalInput")
    xk_d = nc.dram_tensor("xk_t", [NKW, P, KC, 512], F16, kind="ExternalInput")
    xv_d = nc.dram_tensor("xv_t", [LKC, P, KC, P], F16, kind="ExternalInput")
    keep_d = nc.dram_tensor("keep", [P, LKC], F16, kind="ExternalInput")
    wq_d = nc.dram_tensor("wq_t", [P, KC, DH], F16, kind="ExternalInput")
    wk_d = nc.dram_tensor("wk_t", [P, KC, DH], F16, kind="ExternalInput")
    wv_d = nc.dram_tensor("wv_t", [P, KC, DH], F16, kind="ExternalInput")
    wo_d = nc.dram_tensor("wo_t", [P, MC, D], F16, kind="ExternalInput")
    out_d = nc.dram_tensor("outp", [LQ, D], F16, kind="ExternalOutput")

    with tile.TileContext(nc) as tc, nc.allow_low_precision(
        reason="f16 PE matmuls; softmax weights are <=1 and averaged over ~1k keys"
    ), tc.tile_pool(name="persist", bufs=1) as pp:
        # ---------------- persistent SBUF ----------------
        wq_sb = pp.tile([P, KC, DH], F16, tag="wq_sb", name="wq_sb")
        wk_sb = pp.tile([P, KC, DH], F16, tag="wk_sb", name="wk_sb")
        wv_sb = pp.tile([P, KC, DH], F16, tag="wv_sb", name="wv_sb")
        wo_sb = pp.tile([P, MC, D], F16, tag="wo_sb", name="wo_sb")
        xq_sb = pp.tile([P, NQW, KC, 512], F16, tag="xq_sb", name="xq_sb")
        xk_sb = pp.tile([P, NKW, KC, 512], F16, tag="xk_sb", name="xk_sb")
        xv_sb = pp.tile([P, LKC, KC, P], F16, tag="xv_sb", name="xv_sb")
        qt_sb = pp.tile([P, MC, LQ], F16, tag="qt_sb", name="qt_sb")
        kt_sb = pp.tile([P, MC, Lkp], F16, tag="kt_sb", name="kt_sb")
        v_sb = pp.tile([P, LKC, 4 * (HD + 1)], F16, tag="v_sb", name="v_sb")
        ctxt_sb = pp.tile([P, MC, LQ], F16, tag="ctxt_sb", name="ctxt_sb")
        keep_sb = pp.tile([P, LKC], F16, tag="keep_sb", name="keep_sb")

        nc.gpsimd.load_library(library_config.attn)
        nc.sync.dma_start(out=wk_sb[:], in_=wk_d.ap())
        nc.sync.dma_start(out=wq_sb[:], in_=wq_d.ap())
        nc.sync.dma_start(out=wv_sb[:], in_=wv_d.ap())
        nc.sync.dma_start(out=keep_sb[:], in_=keep_d.ap())
        for w in range(NKW):
            nc.sync.dma_start(out=xk_sb[:, w], in_=xk_d.ap()[w])
        for w in range(NQW):
            nc.sync.dma_start(out=xq_sb[:, w], in_=xq_d.ap()[w])
        for lv in range(LKC):
            nc.sync.dma_start(out=xv_sb[:, lv], in_=xv_d.ap()[lv])
        nc.sync.dma_start(out=wo_sb[:], in_=wo_d.ap())

        # ---------------- phase A: projections (weight-stationary) ----------
        with tc.tile_pool(name="pk", bufs=1, space="PSUM") as pk_pool, tc.tile_pool(
            name="pq", bufs=1, space="PSUM"
        ) as pq_pool, tc.tile_pool(name="pv", bufs=2, space="PSUM") as pv_pool:
            # K-projection: kt[m*128+p, l] = sum_d Wk[d, m*128+p] Xk[d, l]
            for m in range(MC):
                psk = pk_pool.tile([P, NKW, 512], F32, tag="psk", name="psk")
                for kc in range(KC):
                    for w in range(NKW):
                        nc.tensor.matmul(
                            psk[:, w, :],
                            wk_sb[:, kc, m * P : (m + 1) * P],
                            xk_sb[:, w, kc, :],
                            start=(kc == 0),
                            stop=(kc == KC - 1),
                        )
                nc.scalar.copy(kt_sb[:, m, :], psk[:])
            # Q-projection, same pattern over 4 windows
            for m in range(MC):
                psq = pq_pool.tile([P, NQW, 512], F32, tag="psq", name="psq")
                for kc in range(KC):
                    for w in range(NQW):
                        nc.tensor.matmul(
                            psq[:, w, :],
                            wq_sb[:, kc, m * P : (m + 1) * P],
                            xq_sb[:, w, kc, :],
                            start=(kc == 0),
                            stop=(kc == KC - 1),
                        )
                nc.vector.tensor_copy(qt_sb[:, m, :], psq[:])
            # V-projection, natural (key-major) layout with fused keep column
            for lv in range(LKC):
                psv = pv_pool.tile([P, DH], F32, tag="psv", name="psv")
                for kc in range(KC):
                    nc.tensor.matmul(
                        psv[:],
                        xv_sb[:, lv, kc, :],
                        wv_sb[:, kc, :],
                        start=(kc == 0),
                        stop=(kc == KC - 1),
                    )
                ev = nc.vector if lv % 2 else nc.scalar
                if ev is nc.vector:
                    nc.vector.tensor_copy(
                        v_sb[:, lv, :].rearrange("p (h c) -> p h c", c=HD + 1)[
                            :, :, 0:HD
                        ],
                        psv[:].rearrange("p (h c) -> p h c", c=HD),
                    )
                else:
                    nc.scalar.copy(
                        v_sb[:, lv, :].rearrange("p (h c) -> p h c", c=HD + 1)[
                            :, :, 0:HD
                        ],
                        psv[:].rearrange("p (h c) -> p h c", c=HD),
                    )
            nc.vector.tensor_copy(
                v_sb[:].rearrange("p l (h c) -> p l h c", c=HD + 1)[:, :, :, HD],
                keep_sb[:, :, None].to_broadcast([P, LKC, 4]),
            )

        # ------------- phase B: attention + folded output projection ---------
        with tc.tile_pool(name="pss", bufs=1, space="PSUM") as pss_pool, tc.tile_pool(
            name="psu", bufs=1, space="PSUM"
        ) as psu_pool, tc.tile_pool(
            name="pso", bufs=1, space="PSUM"
        ) as pso_pool, tc.tile_pool(
            name="expst", bufs=3
        ) as expst_pool, tc.tile_pool(
            name="uhp", bufs=3
        ) as uh_pool, tc.tile_pool(
            name="smal", bufs=3
        ) as small_pool, tc.tile_pool(
            name="ob", bufs=3
        ) as ob_pool:
            for w0 in range(0, LQ, 512):
                cs_w = small_pool.tile([P, MC, 2, 512], F32, tag="cs", name="cs_w")
                uh_tiles = []
                for hp in range(MC):
                    u = psu_pool.tile([P, 2, 512], F32, tag="u", name="u_ps")
                    for pr in range(NPAIR):
                        ps = pss_pool.tile(
                            [P, 2, 2, 512], F32, tag="pss", name="pss_ps"
                        )
                        for j in range(2):
                            lk = 2 * pr + j
                            for hi in range(2):
                                b = HD * hi
                                # S^T[lk block, lq window] = K_h @ Q_h^T
                                # hi=0/1 land on disjoint PE row groups -> the
                                # two matmuls run concurrently in the array
                                nc.tensor.matmul(
                                    ps[:, j, hi, :],
                                    kt_sb[b : b + HD, hp, lk * P : (lk + 1) * P],
                                    qt_sb[b : b + HD, hp, w0 : w0 + 512],
                                    start=True,
                                    stop=True,
                                    tile_position=(b, 0),
                                )
                        expst = expst_pool.tile(
                            [P, 2, 2, 512], F16, tag="expst", name="expst"
                        )
                        nc.scalar.activation(expst[:], ps[:], EXP, scale=SCALE)
                        for j in range(2):
                            lk = 2 * pr + j
                            for hi in range(2):
                                h = 2 * hp + hi
                                # fused ctx+sums: lhsT = [V_h | keep] (M = 65)
                                nc.tensor.matmul(
                                    u[0 : HD + 1, hi, :],
                                    v_sb[:, lk, (HD + 1) * h : (HD + 1) * (h + 1)],
                                    expst[:, j, hi, :],
                                    start=(pr == 0 and j == 0),
                                    stop=(pr == NPAIR - 1 and j == 1),
                                )
                    # evacuate ctx + denominator rows (DVE) so the U banks
                    # free quickly; denominator rows then collect via DMA
                    uh = uh_pool.tile([P, 2, 512], F32, tag="uh", name="uh_sb")
                    uh_tiles.append(uh)
                    nc.vector.tensor_copy(uh[0 : HD + 1, :, :], u[0 : HD + 1, :, :])
                    nc.gpsimd.dma_start(
                        out=cs_w[0:1, hp, 0, :], in_=uh[HD : HD + 1, 0, :]
                    )
                    nc.gpsimd.dma_start(
                        out=cs_w[0:1, hp, 1, :], in_=uh[HD : HD + 1, 1, :]
                    )
                # reciprocal of all 4 denominator vectors at once (~51 ULP)
                dinv = small_pool.tile([P, MC, 2, 512], F32, tag="dinv", name="dinv")
                nc.vector.reciprocal_approx_fast(
                    out=dinv[0:1, :, :, :].rearrange("p a b c -> p (a b c)"),
                    in_=cs_w[0:1, :, :, :].rearrange("p a b c -> p (a b c)"),
                )
                dinv16 = small_pool.tile(
                    [P, MC, 2, 512], F16, tag="dinv16", name="dinv16"
                )
                nc.vector.tensor_copy(dinv16[0:1, :, :, :], dinv[0:1, :, :, :])
                for hp in range(MC):
                    bcr = small_pool.tile([P, 2, 512], F16, tag="bcr", name="bcr")
                    nc.gpsimd.partition_broadcast(
                        bcr[0:HD, :, :], dinv16[0:1, hp, :, :]
                    )
                    nc.vector.tensor_mul(
                        ctxt_sb[0:HD, hp, w0 : w0 + 512],
                        uh_tiles[hp][0:HD, 0, :],
                        bcr[0:HD, 0, :],
                    )
                    # odd head lives on partitions 64:128 of the ctx^T chunk;
                    # DVE cannot shift partitions: multiply at base 0, move
                    # with an SBUF->SBUF DMA
                    ct_o = small_pool.tile([P, 512], F16, tag="cto", name="ct_o")
                    nc.vector.tensor_mul(
                        ct_o[0:HD, :], uh_tiles[hp][0:HD, 1, :], bcr[0:HD, 1, :]
                    )
                    nc.gpsimd.dma_start(
                        out=ctxt_sb[HD:P, hp, w0 : w0 + 512], in_=ct_o[0:HD, :]
                    )
                # ---- output projection for this window ----
                for li, l0 in enumerate(range(w0, w0 + 512, P)):
                    ob = ob_pool.tile([P, D], F16, tag="ob", name="ob_sb")
                    po = pso_pool.tile([P, D], F32, tag="po", name="po_ps")
                    for n0 in range(0, D, 512):
                        for m in range(MC):
                            nc.tensor.matmul(
                                po[:, n0 : n0 + 512],
                                ctxt_sb[:, m, l0 : l0 + P],
                                wo_sb[:, m, n0 : n0 + 512],
                                start=(m == 0),
                                stop=(m == MC - 1),
                            )
                    if li % 2:
                        nc.scalar.copy(ob[:], po[:])
                    else:
                        nc.vector.tensor_copy(ob[:], po[:])
                    nc.sync.dma_start(out=out_d.ap()[l0 : l0 + P, :], in_=ob[:])

    nc.compile()
    nc.m = get_hw_module(nc.m)
    return nc


def _get_nc(Lkp: int):
    if Lkp not in _NC_CACHE:
        _NC_CACHE[Lkp] = _build(Lkp)
    return _NC_CACHE[Lkp]


def _win_layout(x_t, inner):
    """[D, L] -> [L//inner, 128, 8, inner] so each partition's DMA run is contiguous."""
    Ltot = x_t.shape[1]
    return np.ascontiguousarray(
        x_t.reshape(KC, P, Ltot // inner, inner).transpose(2, 1, 0, 3)
    )


def _shard_inputs(query, key, value, mask, Wq, Wk, Wv, Wo):
    B = query.shape[0]
    kept = [np.nonzero(np.asarray(mask[b]) != 0)[0] for b in range(B)]
    lk_max = max((len(k) for k in kept), default=1)
    Lkp = max(512, ((lk_max + 511) // 512) * 512)
    in_maps = []
    for c in range(NCORES):
        b, g = divmod(c, NCORES // B)
        idx = kept[b]
        nk = len(idx)
        xk = np.zeros((D, Lkp), np.float16)
        xv = np.zeros((D, Lkp), np.float16)
        xk[:, :nk] = key[b][idx].T
        xv[:, :nk] = value[b][idx].T
        keepv = np.full((Lkp,), PAD_KEEP, np.float16)
        keepv[:nk] = 1.0
        keepv = np.ascontiguousarray(keepv.reshape(Lkp // P, P).T)
        cols = slice(DH * g, DH * (g + 1))

        def wlay(w):  # [(n p), m] -> [128, n, m]
            return np.ascontiguousarray(
                w.reshape(w.shape[0] // P, P, w.shape[1]).transpose(1, 0, 2).astype(np.float16)
            )

        in_maps.append(
            {
                "xq_t": _win_layout(np.asarray(query[b], np.float32).T.astype(np.float16), 512),
                "xk_t": _win_layout(xk, 512),
                "xv_t": _win_layout(xv, P),
                "keep": keepv,
                "wq_t": wlay(np.asarray(Wq)[cols, :].T.astype(np.float32)),
                "wk_t": wlay(np.asarray(Wk)[cols, :].T.astype(np.float32)),
                "wv_t": wlay(np.asarray(Wv)[cols, :].T.astype(np.float32)),
                "wo_t": wlay(np.asarray(Wo)[:, cols].T.astype(np.float32)),
            }
        )
    return in_maps, Lkp


def kernel(query, key, value, mask, Wq, Wk, Wv, Wo, bo):
    global LAST_RESULTS
    query = np.asarray(query, np.float32)
    key = np.asarray(key, np.float32)
    value = np.asarray(value, np.float32)
    B = query.shape[0]

    in_maps, Lkp = _shard_inputs(query, key, value, mask, Wq, Wk, Wv, Wo)
    nc = _get_nc(Lkp)
    if TRACE:
        _ensure_ntff_hook()
    res = bass_utils.run_bass_kernel_spmd(
        nc, in_maps, list(range(NCORES)), trace=TRACE
    )
    LAST_RESULTS = res

    out = np.zeros((B, LQ, D), np.float32)
    for c in range(NCORES):
        out[c // (NCORES // B)] += res.results[c]["outp"]
    out += np.asarray(bo, np.float32)[None, None, :]
    return out


# revision 8
# speedup vs baseline: 1.3913x; 1.3913x over previous
"""Multi-head attention (B=2, L=2048, d_model=1024, 16 heads) on 8 TRN2 NeuronCores.

Sharding: data-parallel on batch (2) x tensor-parallel on heads (4 groups of 4
heads). Core c handles batch c//4, head group c%4 (Wq/Wk/Wv column-parallel,
Wo row-parallel). Each core emits a partial (2048, 1024) output projection;
the host sums the 4 partials per batch and adds the bias.

Masked keys contribute exactly zero to the reference output (softmax over
-inf), so each data shard compacts K/V to the kept keys (host-side gather,
padded to a multiple of 512; pad slots get zero V rows and ~0 softmax weight).

Device-side single-pass pipeline, paced by ScalarE exp (the hard floor:
8.4M exps/core at 1 elem/cycle/lane):
  K-proj + Q-proj(win0) first so scores start ~10us in; V-proj and the
  remaining Q windows stream inside the window loop as PE filler work.
  Per 512-query window, per head-pair, per 128-key block:
    S^T = K_h Q_h^T (two heads on disjoint PE row groups, concurrent)
    exp on ScalarE (PSUM->SBUF f16), U^T += [V_h | keep]^T @ expS^T (M=65,
    row 64 = softmax denominator).
  Normalize: denominator rows collect onto partitions 0..3 (gpsimd DMA),
  f32 cast + reciprocal_approx_fast + f16 cast on [4,512] (DVE, ~1.4us/win),
  gpsimd partition_broadcast, f16 DVE multiplies -> ctx^T; folded out-proj
  per window -> DMA out. All PSUM evacuations live on DVE; ScalarE runs
  exp only.
"""

import os
import sys

import numpy as np

for _p in ("/opt/trn_rl_repo", "/root/.axon_site/_ro/trn_rl_repo"):
    if os.path.isdir(_p) and _p not in sys.path:
        sys.path.insert(0, _p)

import concourse.bass as bass  # noqa: E402
import concourse.mybir as mybir  # noqa: E402
import concourse.tile as tile  # noqa: E402
from concourse import bacc  # noqa: E402
from concourse import bass_utils  # noqa: E402
from concourse import library_config  # noqa: E402
from concourse.bass_interp import get_hw_module  # noqa: E402

P = 128
D = 1024          # d_model
LQ = 2048         # query length
DH = 256          # per-core head-group width (4 heads x 64)
HD = 64           # head dim
KC = D // P       # contraction chunks for the projections
MC = DH // P      # 2 partition chunks of the head-group dim
NQW = LQ // 512   # query windows
SCALE = 0.125     # 1/sqrt(HD)
F32 = mybir.dt.float32
F16 = mybir.dt.float16
EXP = mybir.ActivationFunctionType.Exp
NCORES = 8
PAD_KEEP = 0.0  # pad keys: V rows zero, keep 0 -> no denominator contribution

_NC_CACHE: dict[int, object] = {}
LAST_RESULTS = None  # test harness reads exec_time_ns off this
TRACE = bool(int(os.environ.get("KERNEL_TRACE", "0")))


def _ensure_ntff_hook():
    """Provide antenv.axon_hooks when the image lacks it (trace-only path)."""
    import importlib
    import types

    try:
        importlib.import_module("antenv.axon_hooks")
        return
    except ImportError:
        pass
    try:
        import antenv
        from trn_agent_boot.trn_boot import _ntff_profile_via_ctypes
    except ImportError:
        return
    mod = types.ModuleType("antenv.axon_hooks")
    state = {"h": None}
    mod.set_axon_ntff_profile_hook = lambda h: state.__setitem__("h", h)
    mod.get_axon_ntff_profile_hook = lambda: state["h"]
    sys.modules["antenv.axon_hooks"] = mod
    antenv.axon_hooks = mod
    so = "/opt/axon/libaxon_pjrt.so"
    if os.path.exists(so):
        mod.set_axon_ntff_profile_hook(_ntff_profile_via_ctypes(so))


def _build(Lkp: int):
    assert Lkp % 512 == 0
    LKC = Lkp // P          # 128-key blocks
    NKW = Lkp // 512        # 512-key windows
    nc = bacc.Bacc(
        "TRN2",
        target_bir_lowering=False,
        debug=False,
        enable_asserts=False,
        num_devices=NCORES,
    )

    xq_d = nc.dram_tensor("xq_t", [NQW, P, KC, 512], F16, kind="ExternalInput")
    xk_d = nc.dram_tensor("xk_t", [NKW, P, KC, 512], F16, kind="ExternalInput")
    xv_d = nc.dram_tensor("xv_t", [LKC, P, KC, P], F16, kind="ExternalInput")
    keep_d = nc.dram_tensor("keep", [P, LKC], F16, kind="ExternalInput")
    wq_d = nc.dram_tensor("wq_t", [P, KC, DH], F16, kind="ExternalInput")
    wk_d = nc.dram_tensor("wk_t", [P, KC, DH], F16, kind="ExternalInput")
    wv_d = nc.dram_tensor("wv_t", [P, KC, DH], F16, kind="ExternalInput")
    wo_d = nc.dram_tensor("wo_t", [P, MC, D], F16, kind="ExternalInput")
    out_d = nc.dram_tensor("outp", [LQ, D], F16, kind="ExternalOutput")

    with tile.TileContext(nc) as tc, nc.allow_low_precision(
        reason="f16 PE matmuls; softmax weights are <=1 and averaged over ~1k keys"
    ), tc.tile_pool(name="persist", bufs=1) as pp:
        # ---------------- persistent SBUF ----------------
        wq_sb = pp.tile([P, KC, DH], F16, tag="wq_sb", name="wq_sb")
        wk_sb = pp.tile([P, KC, DH], F16, tag="wk_sb", name="wk_sb")
        wv_sb = pp.tile([P, KC, DH], F16, tag="wv_sb", name="wv_sb")
        wo_sb = pp.tile([P, MC, D], F16, tag="wo_sb", name="wo_sb")
        xq_sb = pp.tile([P, NQW, KC, 512], F16, tag="xq_sb", name="xq_sb")
        xk_sb = pp.tile([P, NKW, KC, 512], F16, tag="xk_sb", name="xk_sb")
        xv_sb = pp.tile([P, LKC, KC, P], F16, tag="xv_sb", name="xv_sb")
        qt_sb = pp.tile([P, MC, LQ], F16, tag="qt_sb", name="qt_sb")
        kt_sb = pp.tile([P, MC, Lkp], F16, tag="kt_sb", name="kt_sb")
        v_sb = pp.tile([P, LKC, 4 * (HD + 1)], F16, tag="v_sb", name="v_sb")
        ctxt_sb = pp.tile([P, MC, LQ], F16, tag="ctxt_sb", name="ctxt_sb")
        keep_sb = pp.tile([P, LKC], F16, tag="keep_sb", name="keep_sb")

        nc.gpsimd.load_library(library_config.attn)
        # critical-path inputs on the sync queue, V-path on the gpsimd queue
        nc.sync.dma_start(out=wk_sb[:], in_=wk_d.ap())
        nc.sync.dma_start(out=wq_sb[:], in_=wq_d.ap())
        nc.sync.dma_start(out=keep_sb[:], in_=keep_d.ap())
        nc.gpsimd.dma_start(out=wv_sb[:], in_=wv_d.ap())
        for w in range(NKW):
            nc.sync.dma_start(out=xk_sb[:, w], in_=xk_d.ap()[w])
        for lv in range(LKC):
            nc.gpsimd.dma_start(out=xv_sb[:, lv], in_=xv_d.ap()[lv])
        for w in range(NQW):
            nc.sync.dma_start(out=xq_sb[:, w], in_=xq_d.ap()[w])
        nc.sync.dma_start(out=wo_sb[:], in_=wo_d.ap())

        def proj(pool, w_sb, x_sb_w, dst, m):
            """dst[m*128+p, :512] = sum_kc W[kc, m-chunk]^T @ X[kc] (one window)"""
            ps = pool.tile([P, 512], F32, tag="pacc", name="pacc")
            for kc in range(KC):
                nc.tensor.matmul(
                    ps[:],
                    w_sb[:, kc, m * P : (m + 1) * P],
                    x_sb_w[:, kc, :],
                    start=(kc == 0),
                    stop=(kc == KC - 1),
                )
            nc.vector.tensor_copy(dst, ps[:])

        def vproj(pool, lv):
            """v_sb[:, lv] = [Xv[lv] @ Wv^T | keep], key-major with keep col"""
            ps = pool.tile([P, 512], F32, tag="pacc", name="pacc")
            for kc in range(KC):
                nc.tensor.matmul(
                    ps[:, 0:DH],
                    xv_sb[:, lv, kc, :],
                    wv_sb[:, kc, :],
                    start=(kc == 0),
                    stop=(kc == KC - 1),
                )
            nc.vector.tensor_copy(
                v_sb[:, lv, :].rearrange("p (h c) -> p h c", c=HD + 1)[:, :, 0:HD],
                ps[:, 0:DH].rearrange("p (h c) -> p h c", c=HD),
            )

        with tc.tile_pool(name="pacc", bufs=2, space="PSUM") as pacc_pool, tc.tile_pool(
            name="pss", bufs=2, space="PSUM"
        ) as pss_pool, tc.tile_pool(
            name="psu", bufs=1, space="PSUM"
        ) as psu_pool, tc.tile_pool(
            name="expst", bufs=3
        ) as expst_pool, tc.tile_pool(
            name="uhp", bufs=3
        ) as uh_pool, tc.tile_pool(
            name="smal", bufs=3
        ) as small_pool, tc.tile_pool(
            name="ob", bufs=3
        ) as ob_pool:
            # K-proj and Q-proj(win0) first: scores can start once the m=0
            # chunks land; everything else streams in behind.
            for m in range(MC):
                for w in range(NKW):
                    proj(pacc_pool, wk_sb, xk_sb[:, w], kt_sb[:, m, w * 512 : (w + 1) * 512], m)
                proj(pacc_pool, wq_sb, xq_sb[:, 0], qt_sb[:, m, 0:512], m)
            nc.vector.tensor_copy(
                v_sb[:].rearrange("p l (h c) -> p l h c", c=HD + 1)[:, :, :, HD],
                keep_sb[:, :, None].to_broadcast([P, LKC, 4]),
            )

            for iw, w0 in enumerate(range(0, LQ, 512)):
                # ---- PE filler work for this window ----
                if iw == 0:
                    for lv in range(LKC):
                        vproj(pacc_pool, lv)
                if iw + 1 < NQW:
                    for m in range(MC):
                        proj(
                            pacc_pool,
                            wq_sb,
                            xq_sb[:, iw + 1],
                            qt_sb[:, m, w0 + 512 : w0 + 1024],
                            m,
                        )
                # ---- attention ----
                cs16 = small_pool.tile([P, 512], F16, tag="cs16", name="cs16")
                # rows between the 4 collect partitions are never written;
                # memset so the wide reciprocal chain reads defined data
                nc.vector.memset(cs16[0:97, :], 1.0)
                uh_tiles = []
                for hp in range(MC):
                    u = psu_pool.tile([P, 2, 512], F32, tag="u", name="u_ps")
                    for lk in range(LKC):
                        ps = pss_pool.tile([P, 2, 512], F32, tag="pss", name="pss_ps")
                        for hi in range(2):
                            b = HD * hi
                            # S^T[lk block, lq window] = K_h @ Q_h^T; hi=0/1
                            # use disjoint PE row groups -> run concurrently
                            nc.tensor.matmul(
                                ps[:, hi, :],
                                kt_sb[b : b + HD, hp, lk * P : (lk + 1) * P],
                                qt_sb[b : b + HD, hp, w0 : w0 + 512],
                                start=True,
                                stop=True,
                                tile_position=(b, 0),
                            )
                        expst = expst_pool.tile([P, 2, 512], F16, tag="expst", name="expst")
                        nc.scalar.activation(expst[:], ps[:], EXP, scale=SCALE)
                        for hi in range(2):
                            h = 2 * hp + hi
                            # fused ctx+sums: lhsT = [V_h | keep] (M = 65)
                            nc.tensor.matmul(
                                u[0 : HD + 1, hi, :],
                                v_sb[:, lk, (HD + 1) * h : (HD + 1) * (h + 1)],
                                expst[:, hi, :],
                                start=(lk == 0),
                                stop=(lk == LKC - 1),
                            )
                    # evacuate ctx + denominator rows; U banks free fast
                    uh = uh_pool.tile([P, 2, 512], F16, tag="uh", name="uh_sb")
                    uh_tiles.append(uh)
                    nc.vector.tensor_copy(uh[0 : HD + 1, :, :], u[0 : HD + 1, :, :])
                    # denominators of (hp, hi) -> partition 32*(2hp+hi):
                    # engine APs must start at partition 0/32/64/96, and this
                    # lets the reciprocal chain run multi-lane
                    for hi in range(2):
                        k = 32 * (2 * hp + hi)
                        nc.gpsimd.dma_start(
                            out=cs16[k : k + 1, :], in_=uh[HD : HD + 1, hi, :]
                        )
                # reciprocal of the 4 denominator vectors (~51 ULP is plenty)
                cs32 = small_pool.tile([P, 512], F32, tag="cs32", name="cs32")
                dinv32 = small_pool.tile([P, 512], F32, tag="dinv32", name="dinv32")
                dinv16 = small_pool.tile([P, 512], F16, tag="dinv16", name="dinv16")
                # rows 1..31 etc. are unused garbage; DVE cost is
                # free-dim driven so covering 97 rows costs the same
                nc.vector.tensor_copy(cs32[0:97, :], cs16[0:97, :])
                nc.vector.reciprocal_approx_fast(out=dinv32[0:97, :], in_=cs32[0:97, :])
                nc.vector.tensor_copy(dinv16[0:97, :], dinv32[0:97, :])
                # partition_broadcast only reads partition 0 on hardware:
                # bring the three off-zero vectors down with tiny SBUF DMAs
                dz = small_pool.tile([P, 4, 512], F16, tag="dz", name="dz")
                for k in range(4):
                    nc.gpsimd.dma_start(
                        out=dz[0:1, k, :], in_=dinv16[32 * k : 32 * k + 1, :]
                    )
                for hp in range(MC):
                    bcr = small_pool.tile([P, 2, 512], F16, tag="bcr", name="bcr")
                    for hi in range(2):
                        nc.gpsimd.partition_broadcast(
                            bcr[0:HD, hi, :], dz[0:1, 2 * hp + hi, :]
                        )
                    nc.vector.tensor_mul(
                        ctxt_sb[0:HD, hp, w0 : w0 + 512],
                        uh_tiles[hp][0:HD, 0, :],
                        bcr[0:HD, 0, :],
                    )
                    # odd head lives on partitions 64:128 of the ctx^T chunk;
                    # DVE cannot shift partitions: multiply at base 0, move
                    # with an SBUF->SBUF DMA
                    ct_o = small_pool.tile([P, 512], F16, tag="cto", name="ct_o")
                    nc.vector.tensor_mul(
                        ct_o[0:HD, :], uh_tiles[hp][0:HD, 1, :], bcr[0:HD, 1, :]
                    )
                    nc.gpsimd.dma_start(
                        out=ctxt_sb[HD:P, hp, w0 : w0 + 512], in_=ct_o[0:HD, :]
                    )
                # ---- output projection for this window ----
                for l0 in range(w0, w0 + 512, P):
                    ob = ob_pool.tile([P, D], F16, tag="ob", name="ob_sb")
                    for n0 in range(0, D, 512):
                        po = pacc_pool.tile([P, 512], F32, tag="pacc", name="pacc")
                        for m in range(MC):
                            nc.tensor.matmul(
                                po[:],
                                ctxt_sb[:, m, l0 : l0 + P],
                                wo_sb[:, m, n0 : n0 + 512],
                                start=(m == 0),
                                stop=(m == MC - 1),
                            )
                        nc.vector.tensor_copy(ob[:, n0 : n0 + 512], po[:])
                    nc.sync.dma_start(out=out_d.ap()[l0 : l0 + P, :], in_=ob[:])

    nc.compile()
    nc.m = get_hw_module(nc.m)
    return nc


def _get_nc(Lkp: int):
    if Lkp not in _NC_CACHE:
        _NC_CACHE[Lkp] = _build(Lkp)
    return _NC_CACHE[Lkp]


def _win_layout(x_t, inner):
    """[D, L] -> [L//inner, 128, 8, inner] so each partition's DMA run is contiguous."""
    Ltot = x_t.shape[1]
    return np.ascontiguousarray(
        x_t.reshape(KC, P, Ltot // inner, inner).transpose(2, 1, 0, 3)
    )


def _shard_inputs(query, key, value, mask, Wq, Wk, Wv, Wo):
    B = query.shape[0]
    kept = [np.nonzero(np.asarray(mask[b]) != 0)[0] for b in range(B)]
    lk_max = max((len(k) for k in kept), default=1)
    Lkp = max(512, ((lk_max + 511) // 512) * 512)
    in_maps = []
    for c in range(NCORES):
        b, g = divmod(c, NCORES // B)
        idx = kept[b]
        nk = len(idx)
        xk = np.zeros((D, Lkp), np.float16)
        xv = np.zeros((D, Lkp), np.float16)
        xk[:, :nk] = key[b][idx].T
        xv[:, :nk] = value[b][idx].T
        keepv = np.full((Lkp,), PAD_KEEP, np.float16)
        keepv[:nk] = 1.0
        keepv = np.ascontiguousarray(keepv.reshape(Lkp // P, P).T)
        cols = slice(DH * g, DH * (g + 1))

        def wlay(w):  # [(n p), m] -> [128, n, m]
            return np.ascontiguousarray(
                w.reshape(w.shape[0] // P, P, w.shape[1]).transpose(1, 0, 2).astype(np.float16)
            )

        in_maps.append(
            {
                "xq_t": _win_layout(np.asarray(query[b], np.float32).T.astype(np.float16), 512),
                "xk_t": _win_layout(xk, 512),
                "xv_t": _win_layout(xv, P),
                "keep": keepv,
                "wq_t": wlay(np.asarray(Wq)[cols, :].T.astype(np.float32)),
                "wk_t": wlay(np.asarray(Wk)[cols, :].T.astype(np.float32)),
                "wv_t": wlay(np.asarray(Wv)[cols, :].T.astype(np.float32)),
                "wo_t": wlay(np.asarray(Wo)[:, cols].T.astype(np.float32)),
            }
        )
    return in_maps, Lkp


def kernel(query, key, value, mask, Wq, Wk, Wv, Wo, bo):
    global LAST_RESULTS
    query = np.asarray(query, np.float32)
    key = np.asarray(key, np.float32)
    value = np.asarray(value, np.float32)
    B = query.shape[0]

    in_maps, Lkp = _shard_inputs(query, key, value, mask, Wq, Wk, Wv, Wo)
    nc = _get_nc(Lkp)
    if TRACE:
        _ensure_ntff_hook()
    res = bass_utils.run_bass_kernel_spmd(
        nc, in_maps, list(range(NCORES)), trace=TRACE
    )
    LAST_RESULTS = res

    out = np.zeros((B, LQ, D), np.float32)
    for c in range(NCORES):
        out[c // (NCORES // B)] += res.results[c]["outp"]
    out += np.asarray(bo, np.float32)[None, None, :]
    return out
